# revision 20
# baseline (speedup 1.0000x reference)
"""Trainium2 Bass kernel for the point-cloud outlier-removal pipeline
(statistical outlier removal -> curvature filter -> ball-query density filter).

Contract: kernel(x) takes the FULL input x [8, 4096, 3] f32 and returns the
full output (pts_out [8,4096,3] f32, mask3 [8,4096] bool), matching the jax
reference bit-exactly.

Design (data parallel: batch b -> NeuronCore b):
  The pipeline's O(N^2) work (pairwise sq-distances, kNN candidate search,
  ball-query counting) runs on device. The PE computes g = x@x^T per row
  block (fp32, within ~1.5e-6 of the reference's FMA-chain bits), DVE
  combines t = -d = (2g - xx_i) - xx_j with reference-exact rounding, then
  max8/max_index extract per-row top-8 nearest candidates (launches A/B) or
  fused compare+accumulate counts ball-query neighbors (launch C).

  Because the PE's fp32 matmul is not bit-identical to the CPU reference's
  FMA chain, every threshold decision is resolved on the host from exact
  arithmetic: the top-8 candidates are re-scored with bit-exact distances
  (fp64 emulation of fp32 FMA), with a sound fallback to a full-row exact
  recompute whenever the approximate margin cannot guarantee the true top-4
  lies inside the candidate set. Ball-query counts use certain-in /
  possibly-in thresholds (+-B_ERR) with full-row host recount for rows with
  any pair inside the uncertainty band. The per-point 3x3 eigensolve (whose
  smallest eigenvalue is pure LAPACK rounding noise - the covariance of 3
  centered points is rank<=2) and the O(N) statistics run through the exact
  same jax-CPU ops as the reference.
"""
import os
import numpy as np
import jax
import jax.numpy as jnp

import concourse.bass as bass
import concourse.tile as tile
from concourse import mybir, bass2jax
from concourse.bass2jax import _bass_exec_p, partition_id_tensor
from concourse.vector_clock import ScopedClock
from jax.sharding import Mesh, PartitionSpec
from jax.experimental.shard_map import shard_map

# ----------------------------------------------------------------------------
# Workaround: this container's walrus build allows only ONE sync-wait per
# instruction. Split extra waits onto same-engine NoOps / extra drains.
# ----------------------------------------------------------------------------
_orig_lower = tile.TileContext._lower_ordered_insts
_nop_counter = [0]


def _split_waits(ordered):
    for _bb, insts in ordered.items():
        new_list = []
        for inst in insts:
            si = inst.sync_info
            if si is not None and si.on_wait is not None and len(si.on_wait) > 1:
                waits = list(si.on_wait)
                for w in waits[:-1]:
                    _nop_counter[0] += 1
                    nop = mybir.InstNoOp(name=f"waitsplit_nop_{_nop_counter[0]}")
                    nop.engine = inst.engine
                    nop.sync_info = type(si)(on_wait=[w], on_update=[])
                    new_list.append(nop)
                si.on_wait = waits[-1:]
            new_list.append(inst)
        insts[:] = new_list


def _patched_lower(self, ordered):
    _split_waits(ordered)
    return _orig_lower(self, ordered)


def _patched_drain_and_barrier(self, tick_clock, wait_clock):
    nc = self.nc
    drain_inst = nc.sync.drain()
    wait_clock.add_sem_waits(
        drain_inst.ins, ScopedClock({None: tick_clock.global_clock})
    )
    si = drain_inst.ins.sync_info
    waits = list(si.on_wait) if si is not None and si.on_wait else []
    if len(waits) > 1:
        si.on_wait = waits[:1]
        for w in waits[1:]:
            d2 = nc.sync.drain()
            nsi = d2.ins.sync_info
            if nsi is None:
                d2.ins.sync_info = type(si)(on_wait=[w], on_update=[])
            else:
                nsi.on_wait = [w]
    nc.all_engine_barrier()
    assert self.sems is not None
    popped = nc._tile_sem_poison_stack.pop()
    assert popped is self._sem_poison
    nc.clear_and_free_semaphores(list(self.sems.allocated().values()))
    nc.all_engine_barrier()


tile.TileContext._lower_ordered_insts = _patched_lower
tile.TileContext._drain_and_barrier = _patched_drain_and_barrier

# ----------------------------------------------------------------------------
F32 = mybir.dt.float32
U32 = mybir.dt.uint32
f32 = np.float32

NC = 8          # cores == batches
N = 4096
NB = N // 128   # 32 row blocks
NK = N // 512   # 8 col tiles per row block

K_SOR = 3
ALPHA_SOR = 1.1
K_CUR = 3
ALPHA_CUR = 1.8
R1, R2 = 0.08, 0.25
MIN_PTS1, MIN_PTS2 = 3, 20
BIG = f32(1e10)
NEG_BIG = f32(-1e10)
C1 = f32(R1 * R1)   # == f32(0.08)*f32(0.08), checked
C2 = f32(R2 * R2)
B_ERR = 8e-6        # calibrated PE-vs-reference bound (max observed 1.5e-6)

_PROGRAMS = {}
LAST_EXEC_TIMES = []    # per-launch exec_time_ns when NTFF tracing works
LAST_LAUNCH_WALLS = []  # per-launch wall seconds (device exec + dispatch)
_CPU = None


def _cpu():
    global _CPU
    if _CPU is None:
        _CPU = jax.local_devices(backend="cpu")[0]
    return _CPU


# ----------------------------------------------------------------------------
# Device programs
# ----------------------------------------------------------------------------
def _emit_w_block(nc, ps, xTs, hxc, hxrep, w, b):
    """PE+DVE: w = (g - hxx_i) - hxxm_j for row block b.

    w == -d/2 bit-exactly relative to the reference rounding sequence
    (scaling by 2 commutes with round-to-nearest), with masked columns
    forced to exactly -5e9. One STT per 512-col tile (verified on HW to
    round after each ALU stage, matching the reference's two roundings).
    """
    for k in range(NK):
        gp = ps.tile([128, 512], F32, tag="g")
        nc.tensor.matmul(
            gp[:], xTs[:, 128 * b:128 * (b + 1)],
            xTs[:, 512 * k:512 * (k + 1)], start=True, stop=True,
        )
        nc.vector.scalar_tensor_tensor(
            w[:, 512 * k:512 * (k + 1)], gp[:], hxc[:, b:b + 1],
            hxrep[:, 512 * k:512 * (k + 1)],
            op0=mybir.AluOpType.subtract, op1=mybir.AluOpType.subtract,
        )


def _common_prologue(nc, tc, cpool, ps, xT, hxxcol, hxxm, ones):
    xTs = cpool.tile([3, N], F32)
    nc.sync.dma_start(xTs[:], xT[:])
    hxc = cpool.tile([128, NB], F32)
    nc.sync.dma_start(hxc[:], hxxcol[:])
    hxp0 = cpool.tile([1, N], F32)
    nc.sync.dma_start(hxp0[:], hxxm[:])
    onest = cpool.tile([1, 128], F32)
    nc.sync.dma_start(onest[:], ones[:])
    hxrep = cpool.tile([128, N], F32)
    for k in range(NK):
        bp = ps.tile([128, 512], F32, tag="bc")
        nc.tensor.matmul(
            bp[:], onest[:], hxp0[:, 512 * k:512 * (k + 1)],
            start=True, stop=True,
        )
        nc.vector.tensor_copy(hxrep[:, 512 * k:512 * (k + 1)], bp[:])
    return xTs, hxc, hxrep


def _build_top8():
    """w = (g - hxx_i) - hxxm_j; outputs per-row top-8 (values + indices)."""
    nc = bass.Bass("TRN2", target_bir_lowering=False, debug=False, num_devices=NC)
    xT = nc.dram_tensor("xT", [3, N], F32, kind="ExternalInput").ap()
    hxxcol = nc.dram_tensor("hxxcol", [128, NB], F32, kind="ExternalInput").ap()
    hxxm = nc.dram_tensor("hxxm", [1, N], F32, kind="ExternalInput").ap()
    ones = nc.dram_tensor("ones", [1, 128], F32, kind="ExternalInput").ap()
    m8_o = nc.dram_tensor("m8", [N, 8], F32, kind="ExternalOutput").ap()
    i8_o = nc.dram_tensor("i8", [N, 8], U32, kind="ExternalOutput").ap()

    with tile.TileContext(nc) as tc:
        with (
            tc.tile_pool(name="const", bufs=1) as cpool,
            tc.tile_pool(name="ps", bufs=4, space="PSUM") as ps,
            tc.tile_pool(name="work", bufs=2) as work,
            tc.tile_pool(name="small", bufs=3) as small,
        ):
            xTs, hxc, hxrep = _common_prologue(nc, tc, cpool, ps, xT, hxxcol, hxxm, ones)
            for b in range(NB):
                w = work.tile([128, N], F32, tag="w")
                _emit_w_block(nc, ps, xTs, hxc, hxrep, w, b)
                m8 = small.tile([128, 8], F32, tag="m8")
                nc.vector.max(m8[:], w[:])
                i8 = small.tile([128, 8], U32, tag="i8")
                nc.vector.max_index(i8[:], m8[:], w[:])
                nc.sync.dma_start(m8_o[128 * b:128 * (b + 1), :], m8[:])
                nc.sync.dma_start(i8_o[128 * b:128 * (b + 1), :], i8[:])
    return nc


def _build_count():
    """4 fused compare+accumulate counts per row: {r1,r2} x {certain,possible}.

    Thresholds are in w = -d/2 units; w >= u <=> d <= -2u exactly.
    """
    nc = bass.Bass("TRN2", target_bir_lowering=False, debug=False, num_devices=NC)
    xT = nc.dram_tensor("xT", [3, N], F32, kind="ExternalInput").ap()
    hxxcol = nc.dram_tensor("hxxcol", [128, NB], F32, kind="ExternalInput").ap()
    hxxm = nc.dram_tensor("hxxm", [1, N], F32, kind="ExternalInput").ap()
    ones = nc.dram_tensor("ones", [1, 128], F32, kind="ExternalInput").ap()
    cnt_o = nc.dram_tensor("cnt", [N, 4], F32, kind="ExternalOutput").ap()

    bh = B_ERR / 2
    thr = [float(-C1) / 2 + bh, float(-C1) / 2 - bh,
           float(-C2) / 2 + bh, float(-C2) / 2 - bh]

    with tile.TileContext(nc) as tc:
        with (
            tc.tile_pool(name="const", bufs=1) as cpool,
            tc.tile_pool(name="ps", bufs=4, space="PSUM") as ps,
            tc.tile_pool(name="work", bufs=2) as work,
            tc.tile_pool(name="small", bufs=4) as small,
        ):
            xTs, hxc, hxrep = _common_prologue(nc, tc, cpool, ps, xT, hxxcol, hxxm, ones)
            for b in range(NB):
                w = work.tile([128, N], F32, tag="w")
                _emit_w_block(nc, ps, xTs, hxc, hxrep, w, b)
                ind = work.tile([128, N], F32, tag="ind")
                cnts = small.tile([128, 4], F32, tag="cnts")
                for t in range(4):
                    # with accum_out, op1 is the REDUCE op: accum = sum(in >= thr)
                    nc.vector.tensor_scalar(
                        ind[:], w[:], thr[t], 0.0,
                        op0=mybir.AluOpType.is_ge,
                        op1=mybir.AluOpType.add,
                        accum_out=cnts[:, t:t + 1],
                    )
                nc.sync.dma_start(cnt_o[128 * b:128 * (b + 1), :], cnts[:])
    return nc


class _Program:
    """A compiled SPMD program with a persistent jitted callable.

    Rebuilding the jax callable per launch retraces and rehashes the
    multi-MB serialized BIR (~300 ms); building it once drops per-launch
    dispatch to ~10 ms.
    """

    def __init__(self, nc):
        bass2jax.install_neuronx_cc_hook()
        self.nc = nc
        partition_name = (
            nc.partition_id_tensor.name if nc.partition_id_tensor else None
        )
        in_names, out_names, out_avals, zero_outs = [], [], [], []
        for alloc in nc.m.functions[0].allocations:
            if not isinstance(alloc, mybir.MemoryLocationSet):
                continue
            name = alloc.memorylocations[0].name
            if alloc.kind == "ExternalInput":
                if name != partition_name:
                    in_names.append(name)
            elif alloc.kind == "ExternalOutput":
                out_names.append(name)
                shape = tuple(alloc.tensor_shape)
                dtype = mybir.dt.np(alloc.dtype)
                out_avals.append(jax.core.ShapedArray(shape, dtype))
                zero_outs.append(np.zeros((NC * shape[0], *shape[1:]), dtype))
        self.in_names = in_names
        self.out_names = out_names
        self.out_avals = out_avals
        self.zero_outs = zero_outs
        n_params = len(in_names)
        n_outs = len(out_avals)
        all_in_names = list(in_names) + list(out_names)
        if partition_name is not None:
            all_in_names.append(partition_name)

        def _body(*args):
            operands = list(args)
            if partition_name is not None:
                operands.append(partition_id_tensor())
            outs = _bass_exec_p.bind(
                *operands,
                out_avals=tuple(out_avals),
                in_names=tuple(all_in_names),
                out_names=tuple(out_names),
                lowering_input_output_aliases=(),
                sim_require_finite=True,
                sim_require_nnan=True,
                nc=nc,
            )
            return tuple(outs)

        devices = jax.devices()[:NC]
        mesh = Mesh(np.asarray(devices), ("core",))
        self.fn = jax.jit(
            shard_map(
                _body, mesh=mesh,
                in_specs=(PartitionSpec("core"),) * (n_params + n_outs),
                out_specs=(PartitionSpec("core"),) * n_outs,
                check_rep=False,
            ),
            donate_argnums=tuple(range(n_params, n_params + n_outs)),
            keep_unused=True,
        )

    # inputs whose bytes are identical across the launches of one kernel()
    # call - transferred to device once per call via device_cache
    SHARED = ("xT", "hxxcol", "ones")

    def run(self, in_maps, device_cache=None):
        import time as _time
        t0 = _time.monotonic()
        concat_in = []
        for nm in self.in_names:
            if device_cache is not None and nm in device_cache:
                concat_in.append(device_cache[nm])
                continue
            arr = np.concatenate([np.asarray(m[nm]) for m in in_maps], axis=0)
            if device_cache is not None and nm in self.SHARED:
                device_cache[nm] = arr
            concat_in.append(arr)
        outs = self.fn(*concat_in, *[z.copy() for z in self.zero_outs])
        outs = [np.asarray(o) for o in outs]
        LAST_LAUNCH_WALLS.append(_time.monotonic() - t0)
        results = []
        for c in range(NC):
            results.append({
                name: outs[i].reshape(NC, *self.out_avals[i].shape)[c]
                for i, name in enumerate(self.out_names)
            })
        return results


def _programs():
    if not _PROGRAMS:
        _PROGRAMS["top8"] = _Program(_build_top8())
        _PROGRAMS["count"] = _Program(_build_count())
    return _PROGRAMS


def _run(prog, in_maps, device_cache=None):
    return prog.run(in_maps, device_cache)


# ----------------------------------------------------------------------------
# Host-side exact arithmetic (bit-exact emulation of the eager jax reference)
# ----------------------------------------------------------------------------
def _exact_d_rows(x_b64, xx_b, rows):
    """Clamped reference-exact d for full rows. x_b64 [N,3] f64, xx_b [N] f32."""
    a = x_b64[rows][:, None, :]
    bb = x_b64[None, :, :]
    g = (a[..., 0] * bb[..., 0]).astype(f32)
    g = (a[..., 1] * bb[..., 1] + g.astype(np.float64)).astype(f32)
    g = (a[..., 2] * bb[..., 2] + g.astype(np.float64)).astype(f32)
    d = ((xx_b[rows][:, None] - f32(2.0) * g).astype(f32) + xx_b[None, :]).astype(f32)
    return np.maximum(d, f32(0.0))


def _exact_d_cand(x_b64, xx_b, i8):
    """Clamped reference-exact d for per-row candidate lists. i8 [N,8] int."""
    dc = np.empty((N, 8), np.float32)
    for k in range(8):
        j = i8[:, k]
        g = (x_b64[:, 0] * x_b64[j, 0]).astype(f32)
        g = (x_b64[:, 1] * x_b64[j, 1] + g.astype(np.float64)).astype(f32)
        g = (x_b64[:, 2] * x_b64[j, 2] + g.astype(np.float64)).astype(f32)
        dd = ((xx_b - f32(2.0) * g).astype(f32) + xx_b[j]).astype(f32)
        dc[:, k] = np.maximum(dd, f32(0.0))
    return dc


def _topk_exact(d_cand, idx_cand, k):
    """jax.lax.top_k(-d) tie semantics: ascending (d, index)."""
    order = np.lexsort((idx_cand, d_cand), axis=-1)[..., :k]
    return (np.take_along_axis(d_cand, order, -1),
            np.take_along_axis(idx_cand, order, -1))


def _in_maps(x, hxx, hxx_masked):
    maps = []
    ones = np.ones((1, 128), np.float32)
    for i in range(NC):
        maps.append({
            "xT": np.ascontiguousarray(x[i].T),
            "hxxcol": np.ascontiguousarray(hxx[i].reshape(NB, 128).T),
            "hxxm": hxx_masked[i][None, :].copy(),
            "ones": ones,
        })
    return maps


# ----------------------------------------------------------------------------
def kernel(x: np.ndarray):
    x = np.ascontiguousarray(np.asarray(x, dtype=np.float32))
    assert x.shape == (NC, N, 3)
    LAST_EXEC_TIMES.clear()
    LAST_LAUNCH_WALLS.clear()
    progs = _programs()
    x64 = x.astype(np.float64)

    # xx exactly as the eager reference: rounded squares, left-assoc sum
    sq = x * x
    xx = (sq[..., 0] + sq[..., 1]) + sq[..., 2]
    hxx = xx * f32(0.5)           # exact (exponent shift)
    POS_HBIG = f32(5e9)           # masked-column hxx -> w becomes exactly -5e9

    # ---------------- Launch A: unmasked top-8 candidates ----------------
    dev_cache = {}
    resA = _run(progs["top8"], _in_maps(x, hxx, hxx), dev_cache)

    neg_v = np.empty((NC, N, 4), np.float32)
    for b in range(NC):
        i8 = np.asarray(resA[b]["i8"]).astype(np.int64)
        m8 = np.asarray(resA[b]["m8"])
        d_appr8 = -2.0 * m8[:, 7].astype(np.float64)
        dc = _exact_d_cand(x64[b], xx[b], i8)
        dsrt, _ = _topk_exact(dc, i8, 4)
        bad = dsrt[:, 3].astype(np.float64) >= d_appr8 - B_ERR
        # bit-equal approx values make max_index repeat an index, hiding a
        # candidate -> full-row fallback
        bad |= ((m8[:, :7] == m8[:, 1:]) & (i8[:, :7] == i8[:, 1:])).any(1)
        if bad.any():
            rows = np.flatnonzero(bad)
            dfull = _exact_d_rows(x64[b], xx[b], rows)
            idxf = np.broadcast_to(np.arange(N), (len(rows), N))
            dsrt_f, _ = _topk_exact(dfull, idxf, 4)
            dsrt[rows] = dsrt_f
        neg_v[b] = -dsrt

    with jax.default_device(_cpu()):
        v = jnp.mean(-jnp.asarray(neg_v)[..., 1:], axis=-1)
        m = jnp.mean(v, axis=-1, keepdims=True)
        s = jnp.std(v, axis=-1, ddof=1, keepdims=True)
        mask1 = np.asarray((v > m - 0.15 * s) & (v < m + ALPHA_SOR * s))

    # ---------------- Launch B: mask1-masked top-8 candidates ----------------
    hxx_m1 = np.where(mask1, hxx, POS_HBIG).astype(np.float32)
    resB = _run(progs["top8"], _in_maps(x, hxx, hxx_m1), dev_cache)

    nb_idx = np.empty((NC, N, 3), np.int64)
    for b in range(NC):
        i8 = np.asarray(resB[b]["i8"]).astype(np.int64)
        m8 = np.asarray(resB[b]["m8"])
        d_appr8 = -2.0 * m8[:, 7].astype(np.float64)
        dc = _exact_d_cand(x64[b], xx[b], i8)
        dc = np.where(mask1[b][i8], dc, BIG)
        d4, i4 = _topk_exact(dc, i8, 4)
        bad = (d4[:, 3].astype(np.float64) >= d_appr8 - B_ERR) & mask1[b]
        bad |= (i4[:, 0] != np.arange(N)) & mask1[b]
        bad |= ((m8[:, :7] == m8[:, 1:]) & (i8[:, :7] == i8[:, 1:])).any(1) & mask1[b]
        if bad.any():
            rows = np.flatnonzero(bad)
            dfull = _exact_d_rows(x64[b], xx[b], rows)
            dfull = np.where(mask1[b][None, :], dfull, BIG)
            idxf = np.broadcast_to(np.arange(N), (len(rows), N))
            _, i4f = _topk_exact(dfull, idxf, 4)
            i4[rows] = i4f
        nb_idx[b] = i4[:, 1:]
        nb_idx[b][~mask1[b]] = np.array([1, 2, 3])

    with jax.default_device(_cpu()):
        xj = jnp.asarray(x)
        nb_idx_j = jnp.asarray(nb_idx)
        nb = jax.vmap(lambda pts, ids: pts[ids])(xj, nb_idx_j)
        mu = jnp.mean(nb, axis=2, keepdims=True)
        c = nb - mu
        cov = jnp.einsum("bnki,bnkj->bnij", c, c) / (K_CUR - 1)
        ev = jnp.linalg.eigvalsh(cov)
        curv = ev[..., 0] / (jnp.sum(ev, axis=-1) + 1e-6)
        nb_curv = jax.vmap(lambda cc, ids: cc[ids])(curv, nb_idx_j)
        mc = jnp.mean(nb_curv, axis=-1)
        sc = jnp.std(nb_curv, axis=-1)
        mask2 = np.asarray(
            jnp.asarray(mask1) & (curv >= mc - ALPHA_CUR * sc) & (curv <= mc + ALPHA_CUR * sc)
        )

    # ---------------- Launch C: mask2-masked ball-query counts ----------------
    hxx_m2 = np.where(mask2, hxx, POS_HBIG).astype(np.float32)
    resC = _run(progs["count"], _in_maps(x, hxx, hxx_m2), dev_cache)

    cnt1 = np.empty((NC, N), np.int64)
    cnt2 = np.empty((NC, N), np.int64)
    for b in range(NC):
        cnt = np.asarray(resC[b]["cnt"])  # [N,4] f32: in1, hi1, in2, hi2
        cin1, chi1, cin2, chi2 = (cnt[:, t].astype(np.int64) for t in range(4))
        cnt1[b] = cin1
        cnt2[b] = cin2
        rows = np.flatnonzero((chi1 != cin1) | (chi2 != cin2))
        if len(rows):
            dfull = _exact_d_rows(x64[b], xx[b], rows)
            mrow = mask2[b][None, :]
            cnt1[b][rows] = ((dfull <= C1) & mrow).sum(1)
            cnt2[b][rows] = ((dfull <= C2) & mrow).sum(1)

    mask3 = mask2 & (cnt1 >= MIN_PTS1) & (cnt2 > MIN_PTS2)
    pts_out = x * mask3[..., None].astype(x.dtype)
    return pts_out, mask3


# revision 21
# speedup vs baseline: 58.3393x; 58.3393x over previous
"""Trainium2 Bass kernel for the point-cloud outlier-removal pipeline
(statistical outlier removal -> curvature filter -> ball-query density filter).

Contract: kernel(x) takes the FULL input x [8, 4096, 3] f32 and returns the
full output (pts_out [8,4096,3] f32, mask3 [8,4096] bool), matching the jax
reference bit-exactly.

Design (data parallel: batch b -> NeuronCore b):
  The pipeline's O(N^2) work (pairwise sq-distances, kNN candidate search,
  ball-query counting) runs on device. The PE computes g = x@x^T per row
  block (fp32, within ~1.5e-6 of the reference's FMA-chain bits), DVE
  combines t = -d = (2g - xx_i) - xx_j with reference-exact rounding, then
  max8/max_index extract per-row top-8 nearest candidates (launches A/B) or
  fused compare+accumulate counts ball-query neighbors (launch C).

  Because the PE's fp32 matmul is not bit-identical to the CPU reference's
  FMA chain, every threshold decision is resolved on the host from exact
  arithmetic: the top-8 candidates are re-scored with bit-exact distances
  (fp64 emulation of fp32 FMA), with a sound fallback to a full-row exact
  recompute whenever the approximate margin cannot guarantee the true top-4
  lies inside the candidate set. Ball-query counts use certain-in /
  possibly-in thresholds (+-B_ERR) with full-row host recount for rows with
  any pair inside the uncertainty band. The per-point 3x3 eigensolve (whose
  smallest eigenvalue is pure LAPACK rounding noise - the covariance of 3
  centered points is rank<=2) and the O(N) statistics run through the exact
  same jax-CPU ops as the reference.
"""
import numpy as np
import jax
import jax.numpy as jnp

import concourse.bass as bass
import concourse.tile as tile
from concourse import mybir, bass2jax
from concourse.bass2jax import _bass_exec_p, partition_id_tensor
from concourse.vector_clock import ScopedClock
from jax.sharding import Mesh, PartitionSpec
from jax.experimental.shard_map import shard_map

# ----------------------------------------------------------------------------
# Workaround: this container's walrus build allows only ONE sync-wait per
# instruction. Split extra waits onto same-engine NoOps / extra drains.
# ----------------------------------------------------------------------------
_orig_lower = tile.TileContext._lower_ordered_insts
_nop_counter = [0]


def _split_waits(ordered):
    for _bb, insts in ordered.items():
        new_list = []
        for inst in insts:
            si = inst.sync_info
            if si is not None and si.on_wait is not None and len(si.on_wait) > 1:
                waits = list(si.on_wait)
                for w in waits[:-1]:
                    _nop_counter[0] += 1
                    nop = mybir.InstNoOp(name=f"waitsplit_nop_{_nop_counter[0]}")
                    nop.engine = inst.engine
                    nop.sync_info = type(si)(on_wait=[w], on_update=[])
                    new_list.append(nop)
                si.on_wait = waits[-1:]
            new_list.append(inst)
        insts[:] = new_list


def _patched_lower(self, ordered):
    _split_waits(ordered)
    return _orig_lower(self, ordered)


def _patched_drain_and_barrier(self, tick_clock, wait_clock):
    nc = self.nc
    drain_inst = nc.sync.drain()
    wait_clock.add_sem_waits(
        drain_inst.ins, ScopedClock({None: tick_clock.global_clock})
    )
    si = drain_inst.ins.sync_info
    waits = list(si.on_wait) if si is not None and si.on_wait else []
    if len(waits) > 1:
        si.on_wait = waits[:1]
        for w in waits[1:]:
            d2 = nc.sync.drain()
            nsi = d2.ins.sync_info
            if nsi is None:
                d2.ins.sync_info = type(si)(on_wait=[w], on_update=[])
            else:
                nsi.on_wait = [w]
    nc.all_engine_barrier()
    assert self.sems is not None
    popped = nc._tile_sem_poison_stack.pop()
    assert popped is self._sem_poison
    nc.clear_and_free_semaphores(list(self.sems.allocated().values()))
    nc.all_engine_barrier()


tile.TileContext._lower_ordered_insts = _patched_lower
tile.TileContext._drain_and_barrier = _patched_drain_and_barrier

# ----------------------------------------------------------------------------
F32 = mybir.dt.float32
U32 = mybir.dt.uint32
f32 = np.float32

NC = 8          # cores == batches
N = 4096
NB = N // 128   # 32 row blocks
NK = N // 512   # 8 col tiles per row block

K_SOR = 3
ALPHA_SOR = 1.1
K_CUR = 3
ALPHA_CUR = 1.8
R1, R2 = 0.08, 0.25
MIN_PTS1, MIN_PTS2 = 3, 20
BIG = f32(1e10)
C1 = f32(R1 * R1)   # == f32(0.08)*f32(0.08), checked
C2 = f32(R2 * R2)
B_ERR = 8e-6        # calibrated PE-vs-reference bound (max observed 1.5e-6)

_PROGRAMS = {}
LAST_EXEC_TIMES = []    # per-launch exec_time_ns when NTFF tracing works
LAST_LAUNCH_WALLS = []  # per-launch wall seconds (device exec + dispatch)
_CPU = None


def _cpu():
    global _CPU
    if _CPU is None:
        _CPU = jax.local_devices(backend="cpu")[0]
    return _CPU


# ----------------------------------------------------------------------------
# Device programs
# ----------------------------------------------------------------------------
def _emit_w_block(nc, ps, xTs, hxc, hxrep, w, b):
    """PE+DVE: w = (g - hxx_i) - hxxm_j for row block b.

    w == -d/2 bit-exactly relative to the reference rounding sequence
    (scaling by 2 commutes with round-to-nearest), with masked columns
    forced to exactly -5e9. One STT per 512-col tile (verified on HW to
    round after each ALU stage, matching the reference's two roundings).
    """
    for k in range(NK):
        gp = ps.tile([128, 512], F32, tag="g")
        nc.tensor.matmul(
            gp[:], xTs[:, 128 * b:128 * (b + 1)],
            xTs[:, 512 * k:512 * (k + 1)], start=True, stop=True,
        )
        nc.vector.scalar_tensor_tensor(
            w[:, 512 * k:512 * (k + 1)], gp[:], hxc[:, b:b + 1],
            hxrep[:, 512 * k:512 * (k + 1)],
            op0=mybir.AluOpType.subtract, op1=mybir.AluOpType.subtract,
        )


def _common_prologue(nc, tc, cpool, ps, xT, hxxcol, hxxm, ones):
    xTs = cpool.tile([3, N], F32)
    nc.sync.dma_start(xTs[:], xT[:])
    hxc = cpool.tile([128, NB], F32)
    nc.sync.dma_start(hxc[:], hxxcol[:])
    hxp0 = cpool.tile([1, N], F32)
    nc.sync.dma_start(hxp0[:], hxxm[:])
    onest = cpool.tile([1, 128], F32)
    nc.sync.dma_start(onest[:], ones[:])
    hxrep = cpool.tile([128, N], F32)
    for k in range(NK):
        bp = ps.tile([128, 512], F32, tag="bc")
        nc.tensor.matmul(
            bp[:], onest[:], hxp0[:, 512 * k:512 * (k + 1)],
            start=True, stop=True,
        )
        nc.vector.tensor_copy(hxrep[:, 512 * k:512 * (k + 1)], bp[:])
    return xTs, hxc, hxrep


def _build_top8():
    """w = (g - hxx_i) - hxxm_j; outputs per-row top-8 (values + indices)."""
    nc = bass.Bass("TRN2", target_bir_lowering=False, debug=False, num_devices=NC)
    xT = nc.dram_tensor("xT", [3, N], F32, kind="ExternalInput").ap()
    hxxcol = nc.dram_tensor("hxxcol", [128, NB], F32, kind="ExternalInput").ap()
    hxxm = nc.dram_tensor("hxxm", [1, N], F32, kind="ExternalInput").ap()
    ones = nc.dram_tensor("ones", [1, 128], F32, kind="ExternalInput").ap()
    m8_o = nc.dram_tensor("m8", [N, 8], F32, kind="ExternalOutput").ap()
    i8_o = nc.dram_tensor("i8", [N, 8], U32, kind="ExternalOutput").ap()

    with tile.TileContext(nc) as tc:
        with (
            tc.tile_pool(name="const", bufs=1) as cpool,
            tc.tile_pool(name="ps", bufs=4, space="PSUM") as ps,
            tc.tile_pool(name="work", bufs=2) as work,
            tc.tile_pool(name="small", bufs=3) as small,
        ):
            xTs, hxc, hxrep = _common_prologue(nc, tc, cpool, ps, xT, hxxcol, hxxm, ones)
            for b in range(NB):
                w = work.tile([128, N], F32, tag="w")
                _emit_w_block(nc, ps, xTs, hxc, hxrep, w, b)
                m8 = small.tile([128, 8], F32, tag="m8")
                nc.vector.max(m8[:], w[:])
                i8 = small.tile([128, 8], U32, tag="i8")
                nc.vector.max_index(i8[:], m8[:], w[:])
                nc.sync.dma_start(m8_o[128 * b:128 * (b + 1), :], m8[:])
                nc.sync.dma_start(i8_o[128 * b:128 * (b + 1), :], i8[:])
    return nc


def _build_count():
    """4 fused compare+accumulate counts per row: {r1,r2} x {certain,possible}.

    Thresholds are in w = -d/2 units; w >= u <=> d <= -2u exactly.
    """
    nc = bass.Bass("TRN2", target_bir_lowering=False, debug=False, num_devices=NC)
    xT = nc.dram_tensor("xT", [3, N], F32, kind="ExternalInput").ap()
    hxxcol = nc.dram_tensor("hxxcol", [128, NB], F32, kind="ExternalInput").ap()
    hxxm = nc.dram_tensor("hxxm", [1, N], F32, kind="ExternalInput").ap()
    ones = nc.dram_tensor("ones", [1, 128], F32, kind="ExternalInput").ap()
    cnt_o = nc.dram_tensor("cnt", [N, 4], F32, kind="ExternalOutput").ap()

    bh = B_ERR / 2
    thr = [float(-C1) / 2 + bh, float(-C1) / 2 - bh,
           float(-C2) / 2 + bh, float(-C2) / 2 - bh]

    with tile.TileContext(nc) as tc:
        with (
            tc.tile_pool(name="const", bufs=1) as cpool,
            tc.tile_pool(name="ps", bufs=4, space="PSUM") as ps,
            tc.tile_pool(name="work", bufs=2) as work,
            tc.tile_pool(name="small", bufs=4) as small,
        ):
            xTs, hxc, hxrep = _common_prologue(nc, tc, cpool, ps, xT, hxxcol, hxxm, ones)
            for b in range(NB):
                w = work.tile([128, N], F32, tag="w")
                _emit_w_block(nc, ps, xTs, hxc, hxrep, w, b)
                ind = work.tile([128, N], F32, tag="ind")
                cnts = small.tile([128, 4], F32, tag="cnts")
                for t in range(4):
                    # with accum_out, op1 is the REDUCE op: accum = sum(in >= thr)
                    nc.vector.tensor_scalar(
                        ind[:], w[:], thr[t], 0.0,
                        op0=mybir.AluOpType.is_ge,
                        op1=mybir.AluOpType.add,
                        accum_out=cnts[:, t:t + 1],
                    )
                nc.sync.dma_start(cnt_o[128 * b:128 * (b + 1), :], cnts[:])
    return nc


class _Program:
    """A compiled SPMD program with a persistent jitted callable.

    Rebuilding the jax callable per launch retraces and rehashes the
    multi-MB serialized BIR (~300 ms); building it once drops per-launch
    dispatch to ~10 ms.
    """

    def __init__(self, nc):
        bass2jax.install_neuronx_cc_hook()
        self.nc = nc
        partition_name = (
            nc.partition_id_tensor.name if nc.partition_id_tensor else None
        )
        in_names, out_names, out_avals, zero_outs = [], [], [], []
        for alloc in nc.m.functions[0].allocations:
            if not isinstance(alloc, mybir.MemoryLocationSet):
                continue
            name = alloc.memorylocations[0].name
            if alloc.kind == "ExternalInput":
                if name != partition_name:
                    in_names.append(name)
            elif alloc.kind == "ExternalOutput":
                out_names.append(name)
                shape = tuple(alloc.tensor_shape)
                dtype = mybir.dt.np(alloc.dtype)
                out_avals.append(jax.core.ShapedArray(shape, dtype))
                zero_outs.append(np.zeros((NC * shape[0], *shape[1:]), dtype))
        self.in_names = in_names
        self.out_names = out_names
        self.out_avals = out_avals
        self.zero_outs = zero_outs
        n_params = len(in_names)
        n_outs = len(out_avals)
        all_in_names = list(in_names) + list(out_names)
        if partition_name is not None:
            all_in_names.append(partition_name)

        def _body(*args):
            operands = list(args)
            if partition_name is not None:
                operands.append(partition_id_tensor())
            outs = _bass_exec_p.bind(
                *operands,
                out_avals=tuple(out_avals),
                in_names=tuple(all_in_names),
                out_names=tuple(out_names),
                lowering_input_output_aliases=(),
                sim_require_finite=True,
                sim_require_nnan=True,
                nc=nc,
            )
            return tuple(outs)

        devices = jax.devices()[:NC]
        mesh = Mesh(np.asarray(devices), ("core",))
        self.fn = jax.jit(
            shard_map(
                _body, mesh=mesh,
                in_specs=(PartitionSpec("core"),) * (n_params + n_outs),
                out_specs=(PartitionSpec("core"),) * n_outs,
                check_rep=False,
            ),
            donate_argnums=tuple(range(n_params, n_params + n_outs)),
            keep_unused=True,
        )

    # inputs whose bytes are identical across the launches of one kernel()
    # call - transferred to device once per call via device_cache
    SHARED = ("xT", "hxxcol", "ones")

    def run(self, in_maps, device_cache=None):
        import time as _time
        t0 = _time.monotonic()
        concat_in = []
        for nm in self.in_names:
            if device_cache is not None and nm in device_cache:
                concat_in.append(device_cache[nm])
                continue
            arr = np.concatenate([np.asarray(m[nm]) for m in in_maps], axis=0)
            if device_cache is not None and nm in self.SHARED:
                device_cache[nm] = arr
            concat_in.append(arr)
        outs = self.fn(*concat_in, *[z.copy() for z in self.zero_outs])
        outs = [np.asarray(o) for o in outs]
        LAST_LAUNCH_WALLS.append(_time.monotonic() - t0)
        results = []
        for c in range(NC):
            results.append({
                name: outs[i].reshape(NC, *self.out_avals[i].shape)[c]
                for i, name in enumerate(self.out_names)
            })
        return results


def _programs():
    if not _PROGRAMS:
        _PROGRAMS["top8"] = _Program(_build_top8())
        _PROGRAMS["count"] = _Program(_build_count())
    return _PROGRAMS


def _run(prog, in_maps, device_cache=None):
    return prog.run(in_maps, device_cache)


# ----------------------------------------------------------------------------
# Host-side exact arithmetic (bit-exact emulation of the eager jax reference)
# ----------------------------------------------------------------------------
def _exact_d_rows(x_b64, xx_b, rows):
    """Clamped reference-exact d for full rows. x_b64 [N,3] f64, xx_b [N] f32."""
    a = x_b64[rows][:, None, :]
    bb = x_b64[None, :, :]
    g = (a[..., 0] * bb[..., 0]).astype(f32)
    g = (a[..., 1] * bb[..., 1] + g.astype(np.float64)).astype(f32)
    g = (a[..., 2] * bb[..., 2] + g.astype(np.float64)).astype(f32)
    d = ((xx_b[rows][:, None] - f32(2.0) * g).astype(f32) + xx_b[None, :]).astype(f32)
    return np.maximum(d, f32(0.0))


def _exact_d_cand(x_b64, xx_b, i8):
    """Clamped reference-exact d for per-row candidate lists. i8 [N,8] int."""
    dc = np.empty((N, 8), np.float32)
    for k in range(8):
        j = i8[:, k]
        g = (x_b64[:, 0] * x_b64[j, 0]).astype(f32)
        g = (x_b64[:, 1] * x_b64[j, 1] + g.astype(np.float64)).astype(f32)
        g = (x_b64[:, 2] * x_b64[j, 2] + g.astype(np.float64)).astype(f32)
        dd = ((xx_b - f32(2.0) * g).astype(f32) + xx_b[j]).astype(f32)
        dc[:, k] = np.maximum(dd, f32(0.0))
    return dc


def _topk_exact(d_cand, idx_cand, k):
    """jax.lax.top_k(-d) tie semantics: ascending (d, index)."""
    order = np.lexsort((idx_cand, d_cand), axis=-1)[..., :k]
    return (np.take_along_axis(d_cand, order, -1),
            np.take_along_axis(idx_cand, order, -1))


def _in_maps(x, hxx, hxx_masked):
    maps = []
    ones = np.ones((1, 128), np.float32)
    for i in range(NC):
        maps.append({
            "xT": np.ascontiguousarray(x[i].T),
            "hxxcol": np.ascontiguousarray(hxx[i].reshape(NB, 128).T),
            "hxxm": hxx_masked[i][None, :].copy(),
            "ones": ones,
        })
    return maps


# ----------------------------------------------------------------------------
def kernel(x: np.ndarray):
    x = np.ascontiguousarray(np.asarray(x, dtype=np.float32))
    assert x.shape == (NC, N, 3)
    LAST_EXEC_TIMES.clear()
    LAST_LAUNCH_WALLS.clear()
    progs = _programs()
    x64 = x.astype(np.float64)

    # xx exactly as the eager reference: rounded squares, left-assoc sum
    sq = x * x
    xx = (sq[..., 0] + sq[..., 1]) + sq[..., 2]
    hxx = xx * f32(0.5)           # exact (exponent shift)
    POS_HBIG = f32(5e9)           # masked-column hxx -> w becomes exactly -5e9

    # ---------------- Launch A: unmasked top-8 candidates ----------------
    dev_cache = {}
    resA = _run(progs["top8"], _in_maps(x, hxx, hxx), dev_cache)

    neg_v = np.empty((NC, N, 4), np.float32)
    for b in range(NC):
        i8 = np.asarray(resA[b]["i8"]).astype(np.int64)
        m8 = np.asarray(resA[b]["m8"])
        d_appr8 = -2.0 * m8[:, 7].astype(np.float64)
        dc = _exact_d_cand(x64[b], xx[b], i8)
        dsrt, _ = _topk_exact(dc, i8, 4)
        bad = dsrt[:, 3].astype(np.float64) >= d_appr8 - B_ERR
        # bit-equal approx values make max_index repeat an index, hiding a
        # candidate -> full-row fallback
        bad |= ((m8[:, :7] == m8[:, 1:]) & (i8[:, :7] == i8[:, 1:])).any(1)
        if bad.any():
            rows = np.flatnonzero(bad)
            dfull = _exact_d_rows(x64[b], xx[b], rows)
            idxf = np.broadcast_to(np.arange(N), (len(rows), N))
            dsrt_f, _ = _topk_exact(dfull, idxf, 4)
            dsrt[rows] = dsrt_f
        neg_v[b] = -dsrt

    with jax.default_device(_cpu()):
        v = jnp.mean(-jnp.asarray(neg_v)[..., 1:], axis=-1)
        m = jnp.mean(v, axis=-1, keepdims=True)
        s = jnp.std(v, axis=-1, ddof=1, keepdims=True)
        mask1 = np.asarray((v > m - 0.15 * s) & (v < m + ALPHA_SOR * s))

    # ---------------- Launch B: mask1-masked top-8 candidates ----------------
    hxx_m1 = np.where(mask1, hxx, POS_HBIG).astype(np.float32)
    resB = _run(progs["top8"], _in_maps(x, hxx, hxx_m1), dev_cache)

    nb_idx = np.empty((NC, N, 3), np.int64)
    for b in range(NC):
        i8 = np.asarray(resB[b]["i8"]).astype(np.int64)
        m8 = np.asarray(resB[b]["m8"])
        d_appr8 = -2.0 * m8[:, 7].astype(np.float64)
        dc = _exact_d_cand(x64[b], xx[b], i8)
        dc = np.where(mask1[b][i8], dc, BIG)
        d4, i4 = _topk_exact(dc, i8, 4)
        bad = (d4[:, 3].astype(np.float64) >= d_appr8 - B_ERR) & mask1[b]
        bad |= (i4[:, 0] != np.arange(N)) & mask1[b]
        bad |= ((m8[:, :7] == m8[:, 1:]) & (i8[:, :7] == i8[:, 1:])).any(1) & mask1[b]
        if bad.any():
            rows = np.flatnonzero(bad)
            dfull = _exact_d_rows(x64[b], xx[b], rows)
            dfull = np.where(mask1[b][None, :], dfull, BIG)
            idxf = np.broadcast_to(np.arange(N), (len(rows), N))
            _, i4f = _topk_exact(dfull, idxf, 4)
            i4[rows] = i4f
        nb_idx[b] = i4[:, 1:]
        nb_idx[b][~mask1[b]] = np.array([1, 2, 3])

    with jax.default_device(_cpu()):
        xj = jnp.asarray(x)
        nb_idx_j = jnp.asarray(nb_idx)
        nb = jax.vmap(lambda pts, ids: pts[ids])(xj, nb_idx_j)
        mu = jnp.mean(nb, axis=2, keepdims=True)
        c = nb - mu
        cov = jnp.einsum("bnki,bnkj->bnij", c, c) / (K_CUR - 1)
        ev = jnp.linalg.eigvalsh(cov)
        curv = ev[..., 0] / (jnp.sum(ev, axis=-1) + 1e-6)
        nb_curv = jax.vmap(lambda cc, ids: cc[ids])(curv, nb_idx_j)
        mc = jnp.mean(nb_curv, axis=-1)
        sc = jnp.std(nb_curv, axis=-1)
        mask2 = np.asarray(
            jnp.asarray(mask1) & (curv >= mc - ALPHA_CUR * sc) & (curv <= mc + ALPHA_CUR * sc)
        )

    # ---------------- Launch C: mask2-masked ball-query counts ----------------
    hxx_m2 = np.where(mask2, hxx, POS_HBIG).astype(np.float32)
    resC = _run(progs["count"], _in_maps(x, hxx, hxx_m2), dev_cache)

    cnt1 = np.empty((NC, N), np.int64)
    cnt2 = np.empty((NC, N), np.int64)
    for b in range(NC):
        cnt = np.asarray(resC[b]["cnt"])  # [N,4] f32: in1, hi1, in2, hi2
        cin1, chi1, cin2, chi2 = (cnt[:, t].astype(np.int64) for t in range(4))
        cnt1[b] = cin1
        cnt2[b] = cin2
        rows = np.flatnonzero((chi1 != cin1) | (chi2 != cin2))
        if len(rows):
            dfull = _exact_d_rows(x64[b], xx[b], rows)
            mrow = mask2[b][None, :]
            cnt1[b][rows] = ((dfull <= C1) & mrow).sum(1)
            cnt2[b][rows] = ((dfull <= C2) & mrow).sum(1)

    mask3 = mask2 & (cnt1 >= MIN_PTS1) & (cnt2 > MIN_PTS2)
    pts_out = x * mask3[..., None].astype(x.dtype)
    return pts_out, mask3


# revision 24
# speedup vs baseline: 62.9511x; 1.0791x over previous
"""Trainium2 Bass kernel for the point-cloud outlier-removal pipeline
(statistical outlier removal -> curvature filter -> ball-query density filter).

Contract: kernel(x) takes the FULL input x [8, 4096, 3] f32 and returns the
full output (pts_out [8,4096,3] f32, mask3 [8,4096] bool), matching the jax
reference bit-exactly.

Design (data parallel: batch b -> NeuronCore b):
  The pipeline's O(N^2) work (pairwise sq-distances, kNN candidate search,
  ball-query counting) runs on device. The PE computes g = x@x^T per row
  block (fp32, within ~1.5e-6 of the reference's FMA-chain bits), DVE
  combines t = -d = (2g - xx_i) - xx_j with reference-exact rounding, then
  max8/max_index extract per-row top-8 nearest candidates (launches A/B) or
  fused compare+accumulate counts ball-query neighbors (launch C).

  Because the PE's fp32 matmul is not bit-identical to the CPU reference's
  FMA chain, every threshold decision is resolved on the host from exact
  arithmetic: the top-8 candidates are re-scored with bit-exact distances
  (fp64 emulation of fp32 FMA), with a sound fallback to a full-row exact
  recompute whenever the approximate margin cannot guarantee the true top-4
  lies inside the candidate set. Ball-query counts use certain-in /
  possibly-in thresholds (+-B_ERR) with full-row host recount for rows with
  any pair inside the uncertainty band. The per-point 3x3 eigensolve (whose
  smallest eigenvalue is pure LAPACK rounding noise - the covariance of 3
  centered points is rank<=2) and the O(N) statistics run through the exact
  same jax-CPU ops as the reference.
"""
import numpy as np
import jax
import jax.numpy as jnp

import concourse.bass as bass
import concourse.tile as tile
from concourse import mybir, bass2jax
from concourse.bass2jax import _bass_exec_p, partition_id_tensor
from concourse.vector_clock import ScopedClock
from jax.sharding import Mesh, PartitionSpec
from jax.experimental.shard_map import shard_map

# ----------------------------------------------------------------------------
# Workaround: this container's walrus build allows only ONE sync-wait per
# instruction. Split extra waits onto same-engine NoOps / extra drains.
# ----------------------------------------------------------------------------
_orig_lower = tile.TileContext._lower_ordered_insts
_nop_counter = [0]


def _split_waits(ordered):
    for _bb, insts in ordered.items():
        new_list = []
        for inst in insts:
            si = inst.sync_info
            if si is not None and si.on_wait is not None and len(si.on_wait) > 1:
                waits = list(si.on_wait)
                for w in waits[:-1]:
                    _nop_counter[0] += 1
                    nop = mybir.InstNoOp(name=f"waitsplit_nop_{_nop_counter[0]}")
                    nop.engine = inst.engine
                    nop.sync_info = type(si)(on_wait=[w], on_update=[])
                    new_list.append(nop)
                si.on_wait = waits[-1:]
            new_list.append(inst)
        insts[:] = new_list


def _patched_lower(self, ordered):
    _split_waits(ordered)
    return _orig_lower(self, ordered)


def _patched_drain_and_barrier(self, tick_clock, wait_clock):
    nc = self.nc
    drain_inst = nc.sync.drain()
    wait_clock.add_sem_waits(
        drain_inst.ins, ScopedClock({None: tick_clock.global_clock})
    )
    si = drain_inst.ins.sync_info
    waits = list(si.on_wait) if si is not None and si.on_wait else []
    if len(waits) > 1:
        si.on_wait = waits[:1]
        for w in waits[1:]:
            d2 = nc.sync.drain()
            nsi = d2.ins.sync_info
            if nsi is None:
                d2.ins.sync_info = type(si)(on_wait=[w], on_update=[])
            else:
                nsi.on_wait = [w]
    nc.all_engine_barrier()
    assert self.sems is not None
    popped = nc._tile_sem_poison_stack.pop()
    assert popped is self._sem_poison
    nc.clear_and_free_semaphores(list(self.sems.allocated().values()))
    nc.all_engine_barrier()


tile.TileContext._lower_ordered_insts = _patched_lower
tile.TileContext._drain_and_barrier = _patched_drain_and_barrier

# ----------------------------------------------------------------------------
F32 = mybir.dt.float32
U32 = mybir.dt.uint32
f32 = np.float32

NC = 8          # cores == batches
N = 4096
NB = N // 128   # 32 row blocks
NK = N // 512   # 8 col tiles per row block

K_SOR = 3
ALPHA_SOR = 1.1
K_CUR = 3
ALPHA_CUR = 1.8
R1, R2 = 0.08, 0.25
MIN_PTS1, MIN_PTS2 = 3, 20
BIG = f32(1e10)
C1 = f32(R1 * R1)   # == f32(0.08)*f32(0.08), checked
C2 = f32(R2 * R2)
B_ERR = 8e-6        # calibrated PE-vs-reference bound (max observed 1.5e-6)

_PROGRAMS = {}
LAST_EXEC_TIMES = []    # per-launch exec_time_ns when NTFF tracing works
LAST_LAUNCH_WALLS = []  # per-launch wall seconds (device exec + dispatch)
_CPU = None


def _cpu():
    global _CPU
    if _CPU is None:
        _CPU = jax.local_devices(backend="cpu")[0]
    return _CPU


# ----------------------------------------------------------------------------
# Device programs
# ----------------------------------------------------------------------------
def _emit_w_block(nc, ps, xTs, hxc, hxrep, w, b):
    """PE+DVE: w = (g - hxx_i) - hxxm_j for row block b.

    w == -d/2 bit-exactly relative to the reference rounding sequence
    (scaling by 2 commutes with round-to-nearest), with masked columns
    forced to exactly -5e9. One STT per half row (4 PSUM banks), verified on
    HW to round after each ALU stage, matching the reference's two roundings.
    """
    for h in range(2):
        gp = ps.tile([128, 2048], F32, tag="gh")
        for k in range(4):
            nc.tensor.matmul(
                gp[:, 512 * k:512 * (k + 1)], xTs[:, 128 * b:128 * (b + 1)],
                xTs[:, 2048 * h + 512 * k:2048 * h + 512 * (k + 1)],
                start=True, stop=True,
            )
        nc.vector.scalar_tensor_tensor(
            w[:, 2048 * h:2048 * (h + 1)], gp[:], hxc[:, b:b + 1],
            hxrep[:, 2048 * h:2048 * (h + 1)],
            op0=mybir.AluOpType.subtract, op1=mybir.AluOpType.subtract,
        )


def _common_prologue(nc, tc, cpool, ps, xT, hxxcol, hxxm, ones):
    xTs = cpool.tile([3, N], F32)
    nc.sync.dma_start(xTs[:], xT[:])
    hxc = cpool.tile([128, NB], F32)
    nc.sync.dma_start(hxc[:], hxxcol[:])
    hxp0 = cpool.tile([1, N], F32)
    nc.sync.dma_start(hxp0[:], hxxm[:])
    onest = cpool.tile([1, 128], F32)
    nc.sync.dma_start(onest[:], ones[:])
    hxrep = cpool.tile([128, N], F32)
    for k in range(NK):
        bp = ps.tile([128, 512], F32, tag="bc")
        nc.tensor.matmul(
            bp[:], onest[:], hxp0[:, 512 * k:512 * (k + 1)],
            start=True, stop=True,
        )
        nc.vector.tensor_copy(hxrep[:, 512 * k:512 * (k + 1)], bp[:])
    return xTs, hxc, hxrep


def _build_top8():
    """w = (g - hxx_i) - hxxm_j; outputs per-row top-8 (values + indices)."""
    nc = bass.Bass("TRN2", target_bir_lowering=False, debug=False, num_devices=NC)
    xT = nc.dram_tensor("xT", [3, N], F32, kind="ExternalInput").ap()
    hxxcol = nc.dram_tensor("hxxcol", [128, NB], F32, kind="ExternalInput").ap()
    hxxm = nc.dram_tensor("hxxm", [1, N], F32, kind="ExternalInput").ap()
    ones = nc.dram_tensor("ones", [1, 128], F32, kind="ExternalInput").ap()
    m8_o = nc.dram_tensor("m8", [N, 8], F32, kind="ExternalOutput").ap()
    i8_o = nc.dram_tensor("i8", [N, 8], U32, kind="ExternalOutput").ap()

    with tile.TileContext(nc) as tc:
        with tc.tile_pool(name="const", bufs=1) as cpool:
            with tc.tile_pool(name="psb", bufs=2, space="PSUM") as psb:
                xTs, hxc, hxrep = _common_prologue(nc, tc, cpool, psb, xT, hxxcol, hxxm, ones)
            with (
                tc.tile_pool(name="ps", bufs=2, space="PSUM") as ps,
                tc.tile_pool(name="work", bufs=2) as work,
                tc.tile_pool(name="small", bufs=3) as small,
            ):
                for b in range(NB):
                    w = work.tile([128, N], F32, tag="w")
                    _emit_w_block(nc, ps, xTs, hxc, hxrep, w, b)
                    m8 = small.tile([128, 8], F32, tag="m8")
                    nc.vector.max(m8[:], w[:])
                    i8 = small.tile([128, 8], U32, tag="i8")
                    nc.vector.max_index(i8[:], m8[:], w[:])
                    nc.sync.dma_start(m8_o[128 * b:128 * (b + 1), :], m8[:])
                    nc.sync.dma_start(i8_o[128 * b:128 * (b + 1), :], i8[:])
    return nc


def _build_count():
    """4 fused compare+accumulate counts per row: {r1,r2} x {certain,possible}.

    Thresholds are in w = -d/2 units; w >= u <=> d <= -2u exactly.
    """
    nc = bass.Bass("TRN2", target_bir_lowering=False, debug=False, num_devices=NC)
    xT = nc.dram_tensor("xT", [3, N], F32, kind="ExternalInput").ap()
    hxxcol = nc.dram_tensor("hxxcol", [128, NB], F32, kind="ExternalInput").ap()
    hxxm = nc.dram_tensor("hxxm", [1, N], F32, kind="ExternalInput").ap()
    ones = nc.dram_tensor("ones", [1, 128], F32, kind="ExternalInput").ap()
    cnt_o = nc.dram_tensor("cnt", [N, 4], F32, kind="ExternalOutput").ap()

    bh = B_ERR / 2
    thr = [float(-C1) / 2 + bh, float(-C1) / 2 - bh,
           float(-C2) / 2 + bh, float(-C2) / 2 - bh]

    with tile.TileContext(nc) as tc:
        with tc.tile_pool(name="const", bufs=1) as cpool:
            with tc.tile_pool(name="psb", bufs=2, space="PSUM") as psb:
                xTs, hxc, hxrep = _common_prologue(nc, tc, cpool, psb, xT, hxxcol, hxxm, ones)
            with (
                tc.tile_pool(name="ps", bufs=2, space="PSUM") as ps,
                tc.tile_pool(name="work", bufs=2) as work,
                tc.tile_pool(name="small", bufs=4) as small,
            ):
                for b in range(NB):
                    w = work.tile([128, N], F32, tag="w")
                    _emit_w_block(nc, ps, xTs, hxc, hxrep, w, b)
                    ind = work.tile([128, N], F32, tag="ind")
                    cnts = small.tile([128, 4], F32, tag="cnts")
                    for t in range(4):
                        # with accum_out, op1 is the REDUCE op:
                        # accum = sum(in >= thr)
                        nc.vector.tensor_scalar(
                            ind[:], w[:], thr[t], 0.0,
                            op0=mybir.AluOpType.is_ge,
                            op1=mybir.AluOpType.add,
                            accum_out=cnts[:, t:t + 1],
                        )
                    nc.sync.dma_start(cnt_o[128 * b:128 * (b + 1), :], cnts[:])
    return nc


class _Program:
    """A compiled SPMD program with a persistent jitted callable.

    Rebuilding the jax callable per launch retraces and rehashes the
    multi-MB serialized BIR (~300 ms); building it once drops per-launch
    dispatch to ~10 ms.
    """

    def __init__(self, nc):
        bass2jax.install_neuronx_cc_hook()
        self.nc = nc
        partition_name = (
            nc.partition_id_tensor.name if nc.partition_id_tensor else None
        )
        in_names, out_names, out_avals, zero_outs = [], [], [], []
        for alloc in nc.m.functions[0].allocations:
            if not isinstance(alloc, mybir.MemoryLocationSet):
                continue
            name = alloc.memorylocations[0].name
            if alloc.kind == "ExternalInput":
                if name != partition_name:
                    in_names.append(name)
            elif alloc.kind == "ExternalOutput":
                out_names.append(name)
                shape = tuple(alloc.tensor_shape)
                dtype = mybir.dt.np(alloc.dtype)
                out_avals.append(jax.core.ShapedArray(shape, dtype))
                zero_outs.append(np.zeros((NC * shape[0], *shape[1:]), dtype))
        self.in_names = in_names
        self.out_names = out_names
        self.out_avals = out_avals
        self.zero_outs = zero_outs
        n_params = len(in_names)
        n_outs = len(out_avals)
        all_in_names = list(in_names) + list(out_names)
        if partition_name is not None:
            all_in_names.append(partition_name)

        def _body(*args):
            operands = list(args)
            if partition_name is not None:
                operands.append(partition_id_tensor())
            outs = _bass_exec_p.bind(
                *operands,
                out_avals=tuple(out_avals),
                in_names=tuple(all_in_names),
                out_names=tuple(out_names),
                lowering_input_output_aliases=(),
                sim_require_finite=True,
                sim_require_nnan=True,
                nc=nc,
            )
            return tuple(outs)

        devices = jax.devices()[:NC]
        mesh = Mesh(np.asarray(devices), ("core",))
        self.fn = jax.jit(
            shard_map(
                _body, mesh=mesh,
                in_specs=(PartitionSpec("core"),) * (n_params + n_outs),
                out_specs=(PartitionSpec("core"),) * n_outs,
                check_rep=False,
            ),
            donate_argnums=tuple(range(n_params, n_params + n_outs)),
            keep_unused=True,
        )

    # inputs whose bytes are identical across the launches of one kernel()
    # call - transferred to device once per call via device_cache
    SHARED = ("xT", "hxxcol", "ones")

    def run(self, in_maps, device_cache=None):
        import time as _time
        t0 = _time.monotonic()
        concat_in = []
        for nm in self.in_names:
            if device_cache is not None and nm in device_cache:
                concat_in.append(device_cache[nm])
                continue
            arr = np.concatenate([np.asarray(m[nm]) for m in in_maps], axis=0)
            if device_cache is not None and nm in self.SHARED:
                device_cache[nm] = arr
            concat_in.append(arr)
        outs = self.fn(*concat_in, *[z.copy() for z in self.zero_outs])
        outs = [np.asarray(o) for o in outs]
        LAST_LAUNCH_WALLS.append(_time.monotonic() - t0)
        results = []
        for c in range(NC):
            results.append({
                name: outs[i].reshape(NC, *self.out_avals[i].shape)[c]
                for i, name in enumerate(self.out_names)
            })
        return results


def _programs():
    if not _PROGRAMS:
        _PROGRAMS["top8"] = _Program(_build_top8())
        _PROGRAMS["count"] = _Program(_build_count())
    return _PROGRAMS


def _run(prog, in_maps, device_cache=None):
    return prog.run(in_maps, device_cache)


# ----------------------------------------------------------------------------
# Host-side exact arithmetic (bit-exact emulation of the eager jax reference)
# ----------------------------------------------------------------------------
def _exact_d_rows(x_b64, xx_b, rows):
    """Clamped reference-exact d for full rows. x_b64 [N,3] f64, xx_b [N] f32."""
    a = x_b64[rows][:, None, :]
    bb = x_b64[None, :, :]
    g = (a[..., 0] * bb[..., 0]).astype(f32)
    g = (a[..., 1] * bb[..., 1] + g.astype(np.float64)).astype(f32)
    g = (a[..., 2] * bb[..., 2] + g.astype(np.float64)).astype(f32)
    d = ((xx_b[rows][:, None] - f32(2.0) * g).astype(f32) + xx_b[None, :]).astype(f32)
    return np.maximum(d, f32(0.0))


def _exact_d_cand(x_b64, xx_b, i8):
    """Clamped reference-exact d for per-row candidate lists. i8 [N,8] int."""
    dc = np.empty((N, 8), np.float32)
    for k in range(8):
        j = i8[:, k]
        g = (x_b64[:, 0] * x_b64[j, 0]).astype(f32)
        g = (x_b64[:, 1] * x_b64[j, 1] + g.astype(np.float64)).astype(f32)
        g = (x_b64[:, 2] * x_b64[j, 2] + g.astype(np.float64)).astype(f32)
        dd = ((xx_b - f32(2.0) * g).astype(f32) + xx_b[j]).astype(f32)
        dc[:, k] = np.maximum(dd, f32(0.0))
    return dc


def _topk_exact(d_cand, idx_cand, k):
    """jax.lax.top_k(-d) tie semantics: ascending (d, index)."""
    order = np.lexsort((idx_cand, d_cand), axis=-1)[..., :k]
    return (np.take_along_axis(d_cand, order, -1),
            np.take_along_axis(idx_cand, order, -1))


def _in_maps(x, hxx, hxx_masked):
    maps = []
    ones = np.ones((1, 128), np.float32)
    for i in range(NC):
        maps.append({
            "xT": np.ascontiguousarray(x[i].T),
            "hxxcol": np.ascontiguousarray(hxx[i].reshape(NB, 128).T),
            "hxxm": hxx_masked[i][None, :].copy(),
            "ones": ones,
        })
    return maps


# ----------------------------------------------------------------------------
def kernel(x: np.ndarray):
    x = np.ascontiguousarray(np.asarray(x, dtype=np.float32))
    assert x.shape == (NC, N, 3)
    LAST_EXEC_TIMES.clear()
    LAST_LAUNCH_WALLS.clear()
    progs = _programs()
    x64 = x.astype(np.float64)

    # xx exactly as the eager reference: rounded squares, left-assoc sum
    sq = x * x
    xx = (sq[..., 0] + sq[..., 1]) + sq[..., 2]
    hxx = xx * f32(0.5)           # exact (exponent shift)
    POS_HBIG = f32(5e9)           # masked-column hxx -> w becomes exactly -5e9

    # ---------------- Launch A: unmasked top-8 candidates ----------------
    dev_cache = {}
    resA = _run(progs["top8"], _in_maps(x, hxx, hxx), dev_cache)

    neg_v = np.empty((NC, N, 4), np.float32)
    for b in range(NC):
        i8 = np.asarray(resA[b]["i8"]).astype(np.int64)
        m8 = np.asarray(resA[b]["m8"])
        d_appr8 = -2.0 * m8[:, 7].astype(np.float64)
        dc = _exact_d_cand(x64[b], xx[b], i8)
        dsrt, _ = _topk_exact(dc, i8, 4)
        bad = dsrt[:, 3].astype(np.float64) >= d_appr8 - B_ERR
        # bit-equal approx values make max_index repeat an index, hiding a
        # candidate -> full-row fallback
        bad |= ((m8[:, :7] == m8[:, 1:]) & (i8[:, :7] == i8[:, 1:])).any(1)
        if bad.any():
            rows = np.flatnonzero(bad)
            dfull = _exact_d_rows(x64[b], xx[b], rows)
            idxf = np.broadcast_to(np.arange(N), (len(rows), N))
            dsrt_f, _ = _topk_exact(dfull, idxf, 4)
            dsrt[rows] = dsrt_f
        neg_v[b] = -dsrt

    with jax.default_device(_cpu()):
        v = jnp.mean(-jnp.asarray(neg_v)[..., 1:], axis=-1)
        m = jnp.mean(v, axis=-1, keepdims=True)
        s = jnp.std(v, axis=-1, ddof=1, keepdims=True)
        mask1 = np.asarray((v > m - 0.15 * s) & (v < m + ALPHA_SOR * s))

    # ---------------- Launch B: mask1-masked top-8 candidates ----------------
    hxx_m1 = np.where(mask1, hxx, POS_HBIG).astype(np.float32)
    resB = _run(progs["top8"], _in_maps(x, hxx, hxx_m1), dev_cache)

    nb_idx = np.empty((NC, N, 3), np.int64)
    for b in range(NC):
        i8 = np.asarray(resB[b]["i8"]).astype(np.int64)
        m8 = np.asarray(resB[b]["m8"])
        d_appr8 = -2.0 * m8[:, 7].astype(np.float64)
        dc = _exact_d_cand(x64[b], xx[b], i8)
        dc = np.where(mask1[b][i8], dc, BIG)
        d4, i4 = _topk_exact(dc, i8, 4)
        bad = (d4[:, 3].astype(np.float64) >= d_appr8 - B_ERR) & mask1[b]
        bad |= (i4[:, 0] != np.arange(N)) & mask1[b]
        bad |= ((m8[:, :7] == m8[:, 1:]) & (i8[:, :7] == i8[:, 1:])).any(1) & mask1[b]
        if bad.any():
            rows = np.flatnonzero(bad)
            dfull = _exact_d_rows(x64[b], xx[b], rows)
            dfull = np.where(mask1[b][None, :], dfull, BIG)
            idxf = np.broadcast_to(np.arange(N), (len(rows), N))
            _, i4f = _topk_exact(dfull, idxf, 4)
            i4[rows] = i4f
        nb_idx[b] = i4[:, 1:]
        nb_idx[b][~mask1[b]] = np.array([1, 2, 3])

    with jax.default_device(_cpu()):
        xj = jnp.asarray(x)
        nb_idx_j = jnp.asarray(nb_idx)
        nb = jax.vmap(lambda pts, ids: pts[ids])(xj, nb_idx_j)
        mu = jnp.mean(nb, axis=2, keepdims=True)
        c = nb - mu
        cov = jnp.einsum("bnki,bnkj->bnij", c, c) / (K_CUR - 1)
        ev = jnp.linalg.eigvalsh(cov)
        curv = ev[..., 0] / (jnp.sum(ev, axis=-1) + 1e-6)
        nb_curv = jax.vmap(lambda cc, ids: cc[ids])(curv, nb_idx_j)
        mc = jnp.mean(nb_curv, axis=-1)
        sc = jnp.std(nb_curv, axis=-1)
        mask2 = np.asarray(
            jnp.asarray(mask1) & (curv >= mc - ALPHA_CUR * sc) & (curv <= mc + ALPHA_CUR * sc)
        )

    # ---------------- Launch C: mask2-masked ball-query counts ----------------
    hxx_m2 = np.where(mask2, hxx, POS_HBIG).astype(np.float32)
    resC = _run(progs["count"], _in_maps(x, hxx, hxx_m2), dev_cache)

    cnt1 = np.empty((NC, N), np.int64)
    cnt2 = np.empty((NC, N), np.int64)
    for b in range(NC):
        cnt = np.asarray(resC[b]["cnt"])  # [N,4] f32: in1, hi1, in2, hi2
        cin1, chi1, cin2, chi2 = (cnt[:, t].astype(np.int64) for t in range(4))
        cnt1[b] = cin1
        cnt2[b] = cin2
        rows = np.flatnonzero((chi1 != cin1) | (chi2 != cin2))
        if len(rows):
            dfull = _exact_d_rows(x64[b], xx[b], rows)
            mrow = mask2[b][None, :]
            cnt1[b][rows] = ((dfull <= C1) & mrow).sum(1)
            cnt2[b][rows] = ((dfull <= C2) & mrow).sum(1)

    mask3 = mask2 & (cnt1 >= MIN_PTS1) & (cnt2 > MIN_PTS2)
    pts_out = x * mask3[..., None].astype(x.dtype)
    return pts_out, mask3
